# revision 26
# baseline (speedup 1.0000x reference)
"""Trainium2 Bass kernel for a dense recurrent scan (nn_CXBPU_55611236549128).

Math (per timestep t, K=4 microsteps):
    inj  = x_t @ W_in.T + b_in                  scattered into sensory_indices
    h    = relu(h @ W_rec.T + scatter(inj))     microstep 0
    h    = relu(h @ W_rec.T)                    microsteps 1..K-1
    out_t = h[:, output_indices] @ W_out.T + b_out

Sharding: data-parallel over batch, 8 rows per core, W_rec replicated.

Per-core design (feature-major "hT" layout [128 partitions, 16 chunks x 8 batch]):
  - W_rec.T resident in SBUF, streamed as the *moving* matmul operand every
    microstep (h-stationary). The 4 PE column groups (tile_position=(0,32j))
    stream 4 k-tiles concurrently at ~1 moving row/cycle each.
  - Precision: optional W = W1 + W2 fp16 split (npass=2) or single fp16 pass.
  - Fine-grained tiles so the Tile scheduler tracks real deps only:
      * psum: 4 per-bank tiles [128,512] (bank n = output features 512n..)
      * evac: per-bank SBUF fp16 tiles (psum -> SBUF for the transpose-sum)
      * hT:   per-round tiles [128,32] (round r = contraction chunks 4r..4r+3)
    Bank n's results become hT round-tile n for the next microstep.
  - A "transpose-sum" matmul against a 0/1 selector (i128) folds the 4
    partition groups back into feature-major hT (psumT), then relu (+inj).
  - The microstep tail (evac/tmm/relu of late banks) is interleaved into the
    NEXT microstep's main-matmul stream so the PE never waits on DVE/ACT.
  - Readout: 16 tiny matmuls vs scatter-expanded W_out (wsel), deferred.
"""

import os
from contextlib import ExitStack

import numpy as np

N = 2048
B = 64
T = 128
NCORES = 8
BPC = B // NCORES  # 8 batch rows per core
NCHUNK = N // 128  # 16

_CACHE = {}

# 'fp16x2' = two-pass fp16 split (accurate), 'fp16' = single pass (fast)
MODE = os.environ.get("KERNEL_MM_MODE", "fp16")


def _build_nc(n_steps, mode=MODE):
    import concourse.bass as bass
    import concourse.mybir as mybir
    import concourse.tile as tile
    from concourse import bacc

    f32 = mybir.dt.float32
    f16 = mybir.dt.float16
    fmm = f16 if mode.startswith("fp16") else f32
    npass = 2 if mode == "fp16x2" else 1
    nc = bacc.Bacc(trn_type="TRN2")

    wt_d = nc.dram_tensor("wt", [npass * N, N], fmm, kind="ExternalInput")
    injd_d = nc.dram_tensor("injd", [n_steps, 128, 128], f32, kind="ExternalInput")
    wsel_d = nc.dram_tensor("wsel", [128, 2 * NCHUNK], fmm, kind="ExternalInput")
    i128_d = nc.dram_tensor("i128", [128, BPC], fmm, kind="ExternalInput")
    out_d = nc.dram_tensor("out", [2, n_steps * BPC], f32, kind="ExternalOutput")

    NSLAB = npass * NCHUNK

    # evac engines per bank: balance DVE (vector) vs ACT (scalar)
    if npass == 1:
        EVAC_ENG = ["v", "a", "a", "a"]
    else:
        EVAC_ENG = ["v", "a", "v", "a"]

    with tile.TileContext(nc) as tc, ExitStack() as ctx:
        const = ctx.enter_context(tc.tile_pool(name="const", bufs=1))
        hpool = ctx.enter_context(tc.tile_pool(name="h", bufs=8))
        epool = ctx.enter_context(tc.tile_pool(name="evac", bufs=8))
        ipool = ctx.enter_context(tc.tile_pool(name="injd", bufs=2))
        ppool = ctx.enter_context(tc.tile_pool(name="psum", bufs=1, space="PSUM"))
        tpool = ctx.enter_context(tc.tile_pool(name="psumT", bufs=2, space="PSUM"))
        rpool = ctx.enter_context(tc.tile_pool(name="psumR", bufs=2, space="PSUM"))

        # resident W^T slabs: slab u = pass*16 + k-tile at cols [u*2048, ...).
        wt = const.tile([128, NSLAB * N], fmm)
        i128 = const.tile([128, BPC], fmm)
        nc.sync.dma_start(i128[:], i128_d[:])
        wsel = const.tile([128, 2 * NCHUNK], fmm)
        nc.sync.dma_start(wsel[:], wsel_d[:])
        outst = const.tile([2, n_steps * BPC], f32)

        # 4 per-bank psum tiles (1 PSUM bank each). Partitions outside the
        # 4x8 batch groups are never matmul-written; zero them once so the
        # evac's full-partition copy reads zeros there forever.
        psumb = []
        for nb in range(4):
            pb = ppool.tile([128, 512], f32, name=f"psum{nb}")
            nc.vector.memset(pb[:], 0.0)
            psumb.append(pb)

        # hT round-tiles for the fictitious microstep before t=0 (h0 = 0)
        hts = []
        for r in range(4):
            h0 = hpool.tile([128, 32], fmm, name="hT")
            nc.vector.memset(h0[:], 0.0)
            hts.append(h0)

        tc.strict_bb_all_engine_barrier()

        # t=0 injection first: t=0 s=0 needs only it (h0 = 0), and it must
        # not queue behind megabytes of weight slabs.
        injd0 = ipool.tile([128, 128], f32, name="injd")
        nc.sync.dma_start(injd0[:], injd_d[0])

        # Issue the 8/16 MB weight load AFTER the barrier so it overlaps the
        # first timestep. Spread across both HWDGE families + SWDGE; slabs
        # arrive roughly in k-tile order, matching the round order of the
        # first real matmuls.
        for u in range(NSLAB):
            eng = (nc.sync, nc.scalar, nc.gpsimd)[u % 3]
            eng.dma_start(wt[:, u * N : (u + 1) * N], wt_d[u * 128 : (u + 1) * 128, :])

        def emit_main_round(hts, b, r, s):
            # psum[b][32j+bb, f] += sum_k hts[r][k, j*8+bb] * W[kk][k, 512b+f]
            for p in range(npass):
                for j in range(4):
                    kk = 4 * r + j
                    u = p * NCHUNK + kk
                    nc.tensor.matmul(
                        psumb[b][32 * j : 32 * j + BPC, :],
                        lhsT=hts[r][:, j * BPC : (j + 1) * BPC],
                        rhs=wt[:, u * N + 512 * b : u * N + 512 * b + 512],
                        start=(r == 0 and p == 0),
                        stop=(r == 3 and p == npass - 1),
                        tile_position=(0, 32 * j),
                    )

        def emit_evac(evacs, b):
            # halves on DVE+ACT concurrently: latency ~stop+613ns instead of
            # +953ns, which is what makes the 2-flush schedule feasible
            ev = epool.tile([128, 512], fmm, name="evac")
            nc.vector.tensor_copy(ev[:, 0:256], psumb[b][:, 0:256])
            nc.scalar.copy(ev[:, 256:512], psumb[b][:, 256:512])
            evacs[b] = ev

        def emit_tmm(evacs, psumT, b):
            # psumT[m, 32b + c*8 + bb] = sum_j psum[b][32j+bb, c*128+m]
            for c in range(4):
                nc.tensor.matmul(
                    psumT[:, 32 * b + c * BPC : 32 * b + (c + 1) * BPC],
                    lhsT=evacs[b][:, c * 128 : (c + 1) * 128],
                    rhs=i128[:],
                    start=True,
                    stop=True,
                )

        def emit_relu(new_hts, psumT, b, s, injd):
            nh = hpool.tile([128, 32], fmm, name="hT")
            cs = slice(32 * b, 32 * b + 32)
            if s == 0:
                nc.vector.tensor_add(nh[:], psumT[:, cs], injd[:, cs])
                nc.vector.tensor_relu(nh[:], nh[:])
            else:
                nc.vector.tensor_relu(nh[:], psumT[:, cs])
            new_hts[b] = nh

        def make_readout_pieces(hts, t):
            # 4 pieces of 4 chunks; piece i is a CLOSED accumulation group
            # into its own free-column range pr[:, 8i:8i+8] (interleaving an
            # open group with other matmuls fails NEFF load). Emitted inside
            # the tmm flush gaps of the next microstep, where the PE pipeline
            # is drained anyway; a DVE 4-way add then combines the pieces.
            holder = {}

            def piece(i, hts=hts, t=t, holder=holder):
                if i == 0:
                    holder["pr"] = rpool.tile([2, 4 * BPC], f32, name="pr")
                pr = holder["pr"]
                for c in range(4 * i, 4 * i + 4):
                    nc.tensor.matmul(
                        pr[:, i * BPC : (i + 1) * BPC],
                        lhsT=wsel[:, c * 2 : (c + 1) * 2],
                        rhs=hts[c // 4][:, (c % 4) * BPC : (c % 4 + 1) * BPC],
                        start=(c % 4 == 0),
                        stop=(c % 4 == 3),
                    )
                if i == 3:
                    # combine with <=1 PSUM operand per DVE op
                    ot = outst[:, t * BPC : (t + 1) * BPC]
                    nc.vector.tensor_copy(ot, pr[:, 0:BPC])
                    nc.vector.tensor_add(ot, ot, pr[:, BPC : 2 * BPC])
                    nc.vector.tensor_add(ot, ot, pr[:, 2 * BPC : 3 * BPC])
                    nc.vector.tensor_add(ot, ot, pr[:, 3 * BPC : 4 * BPC])

            return [lambda i=i: piece(i) for i in range(4)]

        # tail work deferred from the previous microstep: emitted after this
        # microstep's rounds 0-2 of bank 0 so the PE reaches it late enough
        # that its inputs (evac of the last bank) are long since ready.
        pending = []
        ro_q = []  # readout pieces, drained one per tmm flush gap

        def ro_pop():
            if ro_q:
                ro_q.pop(0)()

        for t in range(n_steps):
            if t == 0:
                injd = injd0
            else:
                injd = ipool.tile([128, 128], f32, name="injd")
                nc.sync.dma_start(injd[:], injd_d[t])
            for s in range(4):
                if t == 0 and s == 0:
                    # h0 = 0, so microstep 0 of t=0 is just relu(injection);
                    # skipping its matmuls also hides the weight-load DMA.
                    new_hts = [None] * 4
                    for b in range(4):
                        nh = hpool.tile([128, 32], fmm, name="hT")
                        nc.vector.tensor_relu(nh[:], injd[:, 32 * b : 32 * b + 32])
                        new_hts[b] = nh
                    hts = new_hts
                    continue
                psumT = tpool.tile([128, NCHUNK * BPC], f32, name="psumT")
                new_hts = [None] * 4
                evacs = [None] * 4

                # microstep top: previous microstep's banks 2,3 fold as ONE
                # merged full-array batch - produces hts[2], hts[3] just in
                # time for bank 0's rounds 2,3 below
                for fn in pending:
                    fn()
                pending = []
                ro_pop()
                ro_pop()
                # all four banks, rounds in order; evac right after each stop
                for b in range(4):
                    for r in range(4):
                        emit_main_round(hts, b, r, s)
                    emit_evac(evacs, b)
                # end batch: banks 0,1 fold (their evacs are long done);
                # feeds the next microstep's early rounds with ~300ns slack
                emit_tmm(evacs, psumT, 0)
                emit_tmm(evacs, psumT, 1)
                ro_pop()
                ro_pop()
                emit_relu(new_hts, psumT, 0, s, injd)
                emit_relu(new_hts, psumT, 1, s, injd)

                pending = [
                    lambda e=evacs, pT=psumT, nh=new_hts, ss=s, ij=injd: (
                        emit_tmm(e, pT, 2),
                        emit_tmm(e, pT, 3),
                        emit_relu(nh, pT, 2, ss, ij),
                        emit_relu(nh, pT, 3, ss, ij),
                    )
                ]
                hts = new_hts

            # readout for timestep t from the final hts of s=3; piece 0 lands
            # behind the pending tmm/relu of bank 3 that completes hts[3]
            assert not ro_q
            ro_q.extend(make_readout_pieces(hts, t))

        for fn in pending:
            fn()
        while ro_q:
            ro_pop()
        nc.sync.dma_start(out_d[:], outst[:])
    nc.compile()
    return nc


def _prep_inputs(inputs, W_rec, W_in, b_in, W_out, sensory_indices, output_indices,
                 n_steps, mode=MODE):
    inputs = np.asarray(inputs, np.float32)
    W_rec = np.asarray(W_rec, np.float32)
    W_in = np.asarray(W_in, np.float32)
    b_in = np.asarray(b_in, np.float32)
    W_out = np.asarray(W_out, np.float32)
    sens = np.asarray(sensory_indices).astype(np.int64)
    oidx = np.asarray(output_indices).astype(np.int64)

    wtf = np.ascontiguousarray(W_rec.T)
    wsel_full = np.zeros((2, N), np.float32)
    np.add.at(wsel_full, (slice(None), oidx), W_out)
    wself = wsel_full.reshape(2, NCHUNK, 128).transpose(2, 1, 0).reshape(128, 2 * NCHUNK)

    if mode.startswith("fp16"):
        w1 = wtf.astype(np.float16)
        if mode == "fp16x2":
            w2 = (wtf - w1.astype(np.float32)).astype(np.float16)
            wt = np.ascontiguousarray(np.concatenate([w1, w2], axis=0))
        else:
            wt = np.ascontiguousarray(w1)
        wsel = np.ascontiguousarray(wself.astype(np.float16))
        i128 = (np.arange(128)[:, None] % 32 == np.arange(BPC)[None, :]).astype(np.float16)
    else:
        wt = wtf
        wsel = np.ascontiguousarray(wself)
        i128 = (np.arange(128)[:, None] % 32 == np.arange(BPC)[None, :]).astype(np.float32)

    # dense injection in hT layout, per core
    inj_all = inputs[:, :n_steps, :] @ W_in.T + b_in  # [B, T, 256]
    inj_dense = np.zeros((B, n_steps, N), np.float32)
    np.add.at(inj_dense, (slice(None), slice(None), sens), inj_all)
    injd_cores = []
    for g in range(NCORES):
        a = inj_dense[g * BPC : (g + 1) * BPC]  # [8, T, 2048]
        a = a.reshape(BPC, n_steps, NCHUNK, 128).transpose(1, 3, 2, 0)
        injd_cores.append(np.ascontiguousarray(a.reshape(n_steps, 128, NCHUNK * BPC)))

    return wt, injd_cores, wsel, i128


def _run(inputs, W_rec, W_in, b_in, W_out, b_out, sensory_indices, output_indices,
         K, n_steps=T, trace=False, mode=MODE):
    from concourse.bass_utils import run_bass_kernel_spmd

    assert int(K) == 4
    wt, injd_cores, wsel, i128 = _prep_inputs(
        inputs, W_rec, W_in, b_in, W_out, sensory_indices, output_indices,
        n_steps, mode)

    key = (n_steps, mode)
    if key not in _CACHE:
        _CACHE[key] = _build_nc(n_steps, mode)
    nc = _CACHE[key]

    in_maps = [
        {"wt": wt, "injd": injd_cores[g], "wsel": wsel, "i128": i128}
        for g in range(NCORES)
    ]
    res = run_bass_kernel_spmd(nc, in_maps, list(range(NCORES)), trace=trace)

    b_out = np.asarray(b_out, np.float32)
    outs = []
    for g in range(NCORES):
        r = np.asarray(res.results[g]["out"])  # [2, T*8]
        outs.append(r.reshape(2, n_steps, BPC).transpose(2, 1, 0))  # [8, T, 2]
    full = np.concatenate(outs, axis=0) + b_out  # [B, T, 2]
    return np.ascontiguousarray(full.astype(np.float32)), res


def kernel(**inputs):
    out, _ = _run(
        inputs["inputs"], inputs["W_rec"], inputs["W_in"], inputs["b_in"],
        inputs["W_out"], inputs["b_out"], inputs["sensory_indices"],
        inputs["output_indices"], inputs["K"],
    )
    return out


# revision 27
# speedup vs baseline: 1.0840x; 1.0840x over previous
"""Trainium2 Bass kernel for a dense recurrent scan (nn_CXBPU_55611236549128).

Math (per timestep t, K=4 microsteps):
    inj  = x_t @ W_in.T + b_in                  scattered into sensory_indices
    h    = relu(h @ W_rec.T + scatter(inj))     microstep 0
    h    = relu(h @ W_rec.T)                    microsteps 1..K-1
    out_t = h[:, output_indices] @ W_out.T + b_out

Sharding: data-parallel over batch, 8 rows per core, W_rec replicated.

Per-core design (feature-major "hT" layout [128 partitions, 16 chunks x 8 batch]):
  - W_rec.T resident in SBUF, streamed as the *moving* matmul operand every
    microstep (h-stationary). The 4 PE column groups (tile_position=(0,32j))
    stream 4 k-tiles concurrently at ~1 moving row/cycle each.
  - Precision: optional W = W1 + W2 fp16 split (npass=2) or single fp16 pass.
  - Fine-grained tiles so the Tile scheduler tracks real deps only:
      * psum: 4 per-bank tiles [128,512] (bank n = output features 512n..)
      * evac: per-bank SBUF fp16 tiles (psum -> SBUF for the transpose-sum)
      * hT:   per-round tiles [128,32] (round r = contraction chunks 4r..4r+3)
    Bank n's results become hT round-tile n for the next microstep.
  - A "transpose-sum" matmul against a 0/1 selector (i128) folds the 4
    partition groups back into feature-major hT (psumT), then relu (+inj).
  - The microstep tail (evac/tmm/relu of late banks) is interleaved into the
    NEXT microstep's main-matmul stream so the PE never waits on DVE/ACT.
  - Readout: 16 tiny matmuls vs scatter-expanded W_out (wsel), deferred.
"""

import os
from contextlib import ExitStack

import numpy as np

N = 2048
B = 64
T = 128
NCORES = 8
BPC = B // NCORES  # 8 batch rows per core
NCHUNK = N // 128  # 16

_CACHE = {}

# 'fp16x2' = two-pass fp16 split (accurate), 'fp16' = single pass (fast)
MODE = os.environ.get("KERNEL_MM_MODE", "fp16")


def _build_nc(n_steps, mode=MODE):
    import concourse.bass as bass
    import concourse.mybir as mybir
    import concourse.tile as tile
    from concourse import bacc

    f32 = mybir.dt.float32
    f16 = mybir.dt.float16
    fmm = f16 if mode.startswith("fp16") else f32
    npass = 2 if mode == "fp16x2" else 1
    nc = bacc.Bacc(trn_type="TRN2")

    wt_d = nc.dram_tensor("wt", [npass * N, N], fmm, kind="ExternalInput")
    injd_d = nc.dram_tensor("injd", [n_steps, 128, 128], f32, kind="ExternalInput")
    wsel_d = nc.dram_tensor("wsel", [128, 2 * NCHUNK], fmm, kind="ExternalInput")
    i128_d = nc.dram_tensor("i128", [128, BPC], fmm, kind="ExternalInput")
    out_d = nc.dram_tensor("out", [2, n_steps * BPC], f32, kind="ExternalOutput")

    NSLAB = npass * NCHUNK

    # evac engines per bank: balance DVE (vector) vs ACT (scalar)
    if npass == 1:
        EVAC_ENG = ["v", "a", "a", "a"]
    else:
        EVAC_ENG = ["v", "a", "v", "a"]

    with tile.TileContext(nc) as tc, ExitStack() as ctx:
        const = ctx.enter_context(tc.tile_pool(name="const", bufs=1))
        hpool = ctx.enter_context(tc.tile_pool(name="h", bufs=8))
        epool = ctx.enter_context(tc.tile_pool(name="evac", bufs=8))
        ipool = ctx.enter_context(tc.tile_pool(name="injd", bufs=2))
        ppool = ctx.enter_context(tc.tile_pool(name="psum", bufs=1, space="PSUM"))
        tpool = ctx.enter_context(tc.tile_pool(name="psumT", bufs=2, space="PSUM"))
        rpool = ctx.enter_context(tc.tile_pool(name="psumR", bufs=2, space="PSUM"))

        # resident W^T slabs: slab u = pass*16 + k-tile at cols [u*2048, ...).
        wt = const.tile([128, NSLAB * N], fmm)
        i128 = const.tile([128, BPC], fmm)
        nc.sync.dma_start(i128[:], i128_d[:])
        wsel = const.tile([128, 2 * NCHUNK], fmm)
        nc.sync.dma_start(wsel[:], wsel_d[:])
        outst = const.tile([2, n_steps * BPC], f32)

        # 4 per-bank psum tiles (1 PSUM bank each). Partitions outside the
        # 4x8 batch groups are never matmul-written; zero them once so the
        # evac's full-partition copy reads zeros there forever.
        psumb = []
        for nb in range(4):
            pb = ppool.tile([128, 512], f32, name=f"psum{nb}")
            nc.vector.memset(pb[:], 0.0)
            psumb.append(pb)

        # hT round-tiles for the fictitious microstep before t=0 (h0 = 0)
        hts = []
        for r in range(4):
            h0 = hpool.tile([128, 32], fmm, name="hT")
            nc.vector.memset(h0[:], 0.0)
            hts.append(h0)

        tc.strict_bb_all_engine_barrier()

        # t=0 injection first: t=0 s=0 needs only it (h0 = 0), and it must
        # not queue behind megabytes of weight slabs.
        injd0 = ipool.tile([128, 128], f32, name="injd")
        nc.sync.dma_start(injd0[:], injd_d[0])

        # Issue the 8/16 MB weight load AFTER the barrier so it overlaps the
        # first timestep. Spread across both HWDGE families + SWDGE; slabs
        # arrive roughly in k-tile order, matching the round order of the
        # first real matmuls.
        for u in range(NSLAB):
            eng = (nc.sync, nc.scalar, nc.gpsimd)[u % 3]
            eng.dma_start(wt[:, u * N : (u + 1) * N], wt_d[u * 128 : (u + 1) * 128, :])

        def emit_main_round(hts, b, r, s):
            # psum[b][32j+bb, f] += sum_k hts[r][k, j*8+bb] * W[kk][k, 512b+f]
            for p in range(npass):
                for j in range(4):
                    kk = 4 * r + j
                    u = p * NCHUNK + kk
                    nc.tensor.matmul(
                        psumb[b][32 * j : 32 * j + BPC, :],
                        lhsT=hts[r][:, j * BPC : (j + 1) * BPC],
                        rhs=wt[:, u * N + 512 * b : u * N + 512 * b + 512],
                        start=(r == 0 and p == 0),
                        stop=(r == 3 and p == npass - 1),
                        tile_position=(0, 32 * j),
                    )

        def emit_evac(evacs, b):
            ev = epool.tile([128, 512], fmm, name="evac")
            if b == 1:
                # halves on DVE+ACT concurrently: bank 1 feeds the merged
                # tmm(0)+tmm(1) batch right after bank 2, deadline-critical
                nc.vector.tensor_copy(ev[:, 0:256], psumb[b][:, 0:256])
                nc.scalar.copy(ev[:, 256:512], psumb[b][:, 256:512])
            elif EVAC_ENG[b] == "v":
                nc.vector.tensor_copy(ev[:], psumb[b][:])
            else:
                nc.scalar.copy(ev[:], psumb[b][:])
            evacs[b] = ev

        def emit_tmm(evacs, psumT, b):
            # psumT[m, 32b + c*8 + bb] = sum_j psum[b][32j+bb, c*128+m]
            for c in range(4):
                nc.tensor.matmul(
                    psumT[:, 32 * b + c * BPC : 32 * b + (c + 1) * BPC],
                    lhsT=evacs[b][:, c * 128 : (c + 1) * 128],
                    rhs=i128[:],
                    start=True,
                    stop=True,
                )

        def emit_relu(new_hts, psumT, b, s, injd):
            nh = hpool.tile([128, 32], fmm, name="hT")
            cs = slice(32 * b, 32 * b + 32)
            if s == 0:
                nc.vector.tensor_add(nh[:], psumT[:, cs], injd[:, cs])
                nc.vector.tensor_relu(nh[:], nh[:])
            else:
                nc.vector.tensor_relu(nh[:], psumT[:, cs])
            new_hts[b] = nh

        def make_readout_pieces(hts, t):
            # 4 pieces of 4 chunks; piece i is a CLOSED accumulation group
            # into its own free-column range pr[:, 8i:8i+8] (interleaving an
            # open group with other matmuls fails NEFF load). Emitted inside
            # the tmm flush gaps of the next microstep, where the PE pipeline
            # is drained anyway; a DVE 4-way add then combines the pieces.
            holder = {}

            def piece(i, hts=hts, t=t, holder=holder):
                if i == 0:
                    holder["pr"] = rpool.tile([2, 4 * BPC], f32, name="pr")
                pr = holder["pr"]
                for c in range(4 * i, 4 * i + 4):
                    nc.tensor.matmul(
                        pr[:, i * BPC : (i + 1) * BPC],
                        lhsT=wsel[:, c * 2 : (c + 1) * 2],
                        rhs=hts[c // 4][:, (c % 4) * BPC : (c % 4 + 1) * BPC],
                        start=(c % 4 == 0),
                        stop=(c % 4 == 3),
                    )
                if i == 3:
                    # combine with <=1 PSUM operand per DVE op
                    ot = outst[:, t * BPC : (t + 1) * BPC]
                    nc.vector.tensor_copy(ot, pr[:, 0:BPC])
                    nc.vector.tensor_add(ot, ot, pr[:, BPC : 2 * BPC])
                    nc.vector.tensor_add(ot, ot, pr[:, 2 * BPC : 3 * BPC])
                    nc.vector.tensor_add(ot, ot, pr[:, 3 * BPC : 4 * BPC])

            return [lambda i=i: piece(i) for i in range(4)]

        # tail work deferred from the previous microstep: emitted after this
        # microstep's rounds 0-2 of bank 0 so the PE reaches it late enough
        # that its inputs (evac of the last bank) are long since ready.
        pending = []
        ro_q = []  # readout pieces, drained one per tmm flush gap

        def ro_pop():
            if ro_q:
                ro_q.pop(0)()

        for t in range(n_steps):
            if t == 0:
                injd = injd0
            else:
                injd = ipool.tile([128, 128], f32, name="injd")
                nc.sync.dma_start(injd[:], injd_d[t])
            for s in range(4):
                if t == 0 and s == 0:
                    # h0 = 0, so microstep 0 of t=0 is just relu(injection);
                    # skipping its matmuls also hides the weight-load DMA.
                    new_hts = [None] * 4
                    for b in range(4):
                        nh = hpool.tile([128, 32], fmm, name="hT")
                        nc.vector.tensor_relu(nh[:], injd[:, 32 * b : 32 * b + 32])
                        new_hts[b] = nh
                    hts = new_hts
                    continue
                psumT = tpool.tile([128, NCHUNK * BPC], f32, name="psumT")
                new_hts = [None] * 4
                evacs = [None] * 4

                # bank 0, rounds 0-2 first (their hT tiles are ready early)
                emit_main_round(hts, 0, 0, s)
                emit_main_round(hts, 0, 1, s)
                emit_main_round(hts, 0, 2, s)
                # previous microstep's tail: tmm+relu of its bank 3 (and a
                # readout piece at timestep boundaries) - produces hts[3]
                for fn in pending:
                    fn()
                pending = []
                ro_pop()
                emit_main_round(hts, 0, 3, s)
                emit_evac(evacs, 0)

                for r in range(4):
                    emit_main_round(hts, 1, r, s)
                emit_evac(evacs, 1)

                for r in range(4):
                    emit_main_round(hts, 2, r, s)
                emit_evac(evacs, 2)
                # merged fold batch for banks 0,1: one full-array flush
                # instead of two; evac(0)/evac(1) are long done by now
                emit_tmm(evacs, psumT, 0)
                emit_tmm(evacs, psumT, 1)
                ro_pop()
                ro_pop()
                emit_relu(new_hts, psumT, 0, s, injd)
                emit_relu(new_hts, psumT, 1, s, injd)

                for r in range(4):
                    emit_main_round(hts, 3, r, s)
                emit_evac(evacs, 3)
                emit_tmm(evacs, psumT, 2)
                ro_pop()
                emit_relu(new_hts, psumT, 2, s, injd)

                pending = [
                    lambda e=evacs, pT=psumT, nh=new_hts, ss=s, ij=injd: (
                        emit_tmm(e, pT, 3),
                        emit_relu(nh, pT, 3, ss, ij),
                    )
                ]
                hts = new_hts

            # readout for timestep t from the final hts of s=3; piece 0 lands
            # behind the pending tmm/relu of bank 3 that completes hts[3]
            assert not ro_q
            ro_q.extend(make_readout_pieces(hts, t))

        for fn in pending:
            fn()
        while ro_q:
            ro_pop()
        nc.sync.dma_start(out_d[:], outst[:])
    nc.compile()
    return nc


def _prep_inputs(inputs, W_rec, W_in, b_in, W_out, sensory_indices, output_indices,
                 n_steps, mode=MODE):
    inputs = np.asarray(inputs, np.float32)
    W_rec = np.asarray(W_rec, np.float32)
    W_in = np.asarray(W_in, np.float32)
    b_in = np.asarray(b_in, np.float32)
    W_out = np.asarray(W_out, np.float32)
    sens = np.asarray(sensory_indices).astype(np.int64)
    oidx = np.asarray(output_indices).astype(np.int64)

    wtf = np.ascontiguousarray(W_rec.T)
    wsel_full = np.zeros((2, N), np.float32)
    np.add.at(wsel_full, (slice(None), oidx), W_out)
    wself = wsel_full.reshape(2, NCHUNK, 128).transpose(2, 1, 0).reshape(128, 2 * NCHUNK)

    if mode.startswith("fp16"):
        w1 = wtf.astype(np.float16)
        if mode == "fp16x2":
            w2 = (wtf - w1.astype(np.float32)).astype(np.float16)
            wt = np.ascontiguousarray(np.concatenate([w1, w2], axis=0))
        else:
            wt = np.ascontiguousarray(w1)
        wsel = np.ascontiguousarray(wself.astype(np.float16))
        i128 = (np.arange(128)[:, None] % 32 == np.arange(BPC)[None, :]).astype(np.float16)
    else:
        wt = wtf
        wsel = np.ascontiguousarray(wself)
        i128 = (np.arange(128)[:, None] % 32 == np.arange(BPC)[None, :]).astype(np.float32)

    # dense injection in hT layout, per core
    inj_all = inputs[:, :n_steps, :] @ W_in.T + b_in  # [B, T, 256]
    inj_dense = np.zeros((B, n_steps, N), np.float32)
    np.add.at(inj_dense, (slice(None), slice(None), sens), inj_all)
    injd_cores = []
    for g in range(NCORES):
        a = inj_dense[g * BPC : (g + 1) * BPC]  # [8, T, 2048]
        a = a.reshape(BPC, n_steps, NCHUNK, 128).transpose(1, 3, 2, 0)
        injd_cores.append(np.ascontiguousarray(a.reshape(n_steps, 128, NCHUNK * BPC)))

    return wt, injd_cores, wsel, i128


def _run(inputs, W_rec, W_in, b_in, W_out, b_out, sensory_indices, output_indices,
         K, n_steps=T, trace=False, mode=MODE):
    from concourse.bass_utils import run_bass_kernel_spmd

    assert int(K) == 4
    wt, injd_cores, wsel, i128 = _prep_inputs(
        inputs, W_rec, W_in, b_in, W_out, sensory_indices, output_indices,
        n_steps, mode)

    key = (n_steps, mode)
    if key not in _CACHE:
        _CACHE[key] = _build_nc(n_steps, mode)
    nc = _CACHE[key]

    in_maps = [
        {"wt": wt, "injd": injd_cores[g], "wsel": wsel, "i128": i128}
        for g in range(NCORES)
    ]
    res = run_bass_kernel_spmd(nc, in_maps, list(range(NCORES)), trace=trace)

    b_out = np.asarray(b_out, np.float32)
    outs = []
    for g in range(NCORES):
        r = np.asarray(res.results[g]["out"])  # [2, T*8]
        outs.append(r.reshape(2, n_steps, BPC).transpose(2, 1, 0))  # [8, T, 2]
    full = np.concatenate(outs, axis=0) + b_out  # [B, T, 2]
    return np.ascontiguousarray(full.astype(np.float32)), res


def kernel(**inputs):
    out, _ = _run(
        inputs["inputs"], inputs["W_rec"], inputs["W_in"], inputs["b_in"],
        inputs["W_out"], inputs["b_out"], inputs["sensory_indices"],
        inputs["output_indices"], inputs["K"],
    )
    return out
